# revision 4
# baseline (speedup 1.0000x reference)
"""Trainium2 Bass kernel for MeshNN 1-D FEM interpolation (nn_MeshNN dense_mlp).

Math: the reference builds a dense [N, 510] hat-function matrix and does a
matvec, plus Dirichlet boundary hats.  On the uniform grid every evaluation
point x lies in exactly one cell [c_k, c_{k+1}] and only the two hats centered
at c_k and c_{k+1} are nonzero there, with values (1-t) and t,
t = x/h - k, k = floor(x/h).  With w = [dd0, nodal_values..., ddL] (hat
coefficients indexed by center node) the result is exactly

    u(x) = w[k] + t*(w[k+1] - w[k])

so the per-point work is one 8-byte table lookup plus a 3-op lerp -- no dense
[N, 512] intermediate at all.

Device mapping (data-parallel across 8 NeuronCores, 16384 points each; inside
each NC the 8 GPSIMD Q7 cores each own 2048 points):
  1. x loaded twice: "wrapped" [128, 128] layout for index computation
     (point j*2048 + s*16 + q at partition 16j+q, column s -- the layout
     ap_gather wants its per-core index stream in), and "replicated" layout
     (core j's 2048 points along the free dim of partition 16j).
  2. k = floor(x*inv_h) via t = mod(y, 1); k = y - t (rounding-mode free),
     cast to int16.
  3. GPSIMD ap_gather pulls (w[k], dw[k]) pairs from a per-partition copy of
     the 512x2 table (SBUF-local gather, 4 chunks for pipelining).
  4. DVE lerp u = w0 + t*dw in the replicated layout; one DMA stores each
     Q7 core's row back to DRAM.
The (w, dw) table is host-marshalled from the small replicated input vectors
(O(n_nodes) restacking, no O(N) host work).
"""

import os

import numpy as np

import concourse.bacc as bacc
import concourse.bass as bass
import concourse.tile as tile
from concourse import mybir
from concourse.bass_utils import run_bass_kernel_spmd

N_POINTS = 131072
N_NODES = 512
N_CORES = 8
PPC = N_POINTS // N_CORES   # points per NeuronCore = 16384
P = 128                     # SBUF partitions
PPQ = PPC // 8              # points per Q7 core = 2048
N_CHUNKS = 4
IC = PPQ // N_CHUNKS        # gather idxs per chunk per Q7 core = 512


def _groups(ap):
    """[128, F] tile AP -> [8, F] view on partitions {0,16,...,112}."""
    return ap.rearrange("(j r) f -> j r f", r=16)[:, 0, :]


def _build_nc(inv_h: float):
    nc = bacc.Bacc("TRN2", target_bir_lowering=False, debug=False,
                   num_devices=N_CORES)
    x_d = nc.dram_tensor("x", [PPC], mybir.dt.float32,
                         kind="ExternalInput").ap()
    wp_d = nc.dram_tensor("wp", [N_NODES, 2], mybir.dt.float32,
                          kind="ExternalInput").ap()
    out_d = nc.dram_tensor("out", [PPC], mybir.dt.float32,
                           kind="ExternalOutput").ap()

    with tile.TileContext(nc) as tc:
        with tc.tile_pool(name="sb", bufs=1) as pool:
            # (w, dw) table broadcast to every partition
            tabsb = pool.tile([P, N_NODES * 2], mybir.dt.float32, tag="tab")
            nc.sync.dma_start(
                tabsb[:],
                wp_d.rearrange("n d -> (n d)")[None, :]
                    .to_broadcast([P, N_NODES * 2]),
            )
            # replicated layout: core j's points on partition 16j
            x2 = pool.tile([P, PPQ], mybir.dt.float32, tag="x2")
            nc.sync.dma_start(
                x2[:].rearrange("(j r) f -> j r f", r=16),
                x_d.rearrange("(j f) -> j f", j=8)[:, None, :]
                   .to_broadcast([8, 16, PPQ]))

            y2 = pool.tile([P, PPQ], mybir.dt.float32, tag="y2")
            nc.scalar.activation(y2[:], x2[:],
                                 mybir.ActivationFunctionType.Copy,
                                 scale=inv_h)
            MAGIC = 8388608.0  # 2^23: adding+subtracting rounds f32 to int
            r1 = pool.tile([P, PPQ], mybir.dt.float32, tag="r1")
            nc.scalar.activation(r1[:], x2[:],
                                 mybir.ActivationFunctionType.Copy,
                                 bias=MAGIC, scale=inv_h)
            r = pool.tile([P, PPQ], mybir.dt.float32, tag="r")
            nc.vector.tensor_scalar(r[:], r1[:], MAGIC, None,
                                    mybir.AluOpType.subtract)
            cmp = pool.tile([P, PPQ], mybir.dt.float32, tag="cmp")
            nc.vector.tensor_tensor(cmp[:], r[:], y2[:],
                                    mybir.AluOpType.is_gt)
            t0 = pool.tile([P, PPQ], mybir.dt.float32, tag="t0")
            nc.vector.tensor_sub(t0[:], y2[:], r[:])
            t = pool.tile([P, PPQ], mybir.dt.float32, tag="t")
            nc.vector.tensor_add(t[:], t0[:], cmp[:])

            # wrapped layout for the gather's index stream
            xw = pool.tile([P, PPQ // 16], mybir.dt.float32, tag="xw")
            for j in range(8):
                nc.sync.dma_start(
                    xw[16 * j:16 * (j + 1), :],
                    x_d[j * PPQ:(j + 1) * PPQ].rearrange("(s q) -> q s", q=16))
            yw = pool.tile([P, PPQ // 16], mybir.dt.float32, tag="yw")
            nc.scalar.activation(yw[:], xw[:],
                                 mybir.ActivationFunctionType.Copy,
                                 scale=inv_h)
            rw1 = pool.tile([P, PPQ // 16], mybir.dt.float32, tag="rw1")
            nc.scalar.activation(rw1[:], xw[:],
                                 mybir.ActivationFunctionType.Copy,
                                 bias=MAGIC, scale=inv_h)
            rw = pool.tile([P, PPQ // 16], mybir.dt.float32, tag="rw")
            nc.vector.tensor_scalar(rw[:], rw1[:], MAGIC, None,
                                    mybir.AluOpType.subtract)
            cw = pool.tile([P, PPQ // 16], mybir.dt.float32, tag="cw")
            nc.vector.tensor_tensor(cw[:], rw[:], yw[:],
                                    mybir.AluOpType.is_gt)
            kwf = pool.tile([P, PPQ // 16], mybir.dt.float32, tag="kwf")
            nc.vector.tensor_sub(kwf[:], rw[:], cw[:])
            kw = pool.tile([P, PPQ // 16], mybir.dt.int16, tag="kw")
            nc.vector.tensor_copy(kw[:], kwf[:])

            u = pool.tile([P, PPQ], mybir.dt.float32, tag="u")
            for c in range(N_CHUNKS):
                isl = slice(c * IC, (c + 1) * IC)
                ssl = slice(c * (IC // 16), (c + 1) * (IC // 16))
                g = pool.tile([P, IC * 2], mybir.dt.float32, tag=f"g{c % 2}")
                nc.gpsimd.ap_gather(
                    out_ap=g[:].rearrange("p (i d) -> p i d", d=2),
                    in_ap=tabsb[:].rearrange("p (n d) -> p n d", d=2),
                    idxs_ap=kw[:, ssl],
                    channels=P,
                    num_elems=N_NODES,
                    d=2,
                    num_idxs=IC,
                )
                g3 = g[:].rearrange("p (i d) -> p i d", d=2)
                w0 = g3[:, :, 0]
                dw = g3[:, :, 1]
                m = pool.tile([P, IC], mybir.dt.float32, tag=f"m{c % 2}")
                nc.vector.tensor_mul(m[:], t[:, isl], dw)
                nc.vector.tensor_add(u[:, isl], m[:], w0)

            nc.sync.dma_start(out_d.rearrange("(j f) -> j f", j=8),
                              _groups(u[:]))

    nc.compile()
    return nc


_CACHE = {}


def kernel(x, coordinates, nodal_values, dd_weights):
    x = np.asarray(x, dtype=np.float32)
    c = np.asarray(coordinates, dtype=np.float32)
    v = np.asarray(nodal_values, dtype=np.float32)
    dd = np.asarray(dd_weights, dtype=np.float32)

    # hat coefficients by center node: [dd0, v..., ddL]; table rows (w, dw)
    w = np.empty(N_NODES, dtype=np.float32)
    w[0] = dd[0]
    w[1:N_NODES - 1] = v
    w[N_NODES - 1] = dd[1]
    wp = np.zeros((N_NODES, 2), dtype=np.float32)
    wp[:, 0] = w
    wp[:N_NODES - 1, 1] = w[1:] - w[:N_NODES - 1]

    inv_h = float(1.0 / (np.float32(c[1]) - np.float32(c[0])))

    if "nc" not in _CACHE:
        _CACHE["nc"] = _build_nc(inv_h)
    nc = _CACHE["nc"]

    chunks = x.reshape(N_CORES, PPC)
    in_maps = [{"x": np.ascontiguousarray(chunks[i]), "wp": wp}
               for i in range(N_CORES)]

    trace = bool(int(os.environ.get("BASS_KERNEL_TRACE", "0")))
    if trace:
        _install_ntff_hook()
    res = run_bass_kernel_spmd(nc, in_maps, list(range(N_CORES)), trace=trace)
    if trace:
        _CACHE["exec_time_ns"] = res.exec_time_ns
        _CACHE["profile_json"] = res.profile_json
        _CACHE["trace"] = res.instructions_and_trace

    out = np.concatenate([res.results[i]["out"] for i in range(N_CORES)])
    return out.reshape(N_POINTS, 1)


def _install_ntff_hook():
    """Shim antenv.axon_hooks + NTFF ctypes hook so trace=True works under
    axon in this container (normally installed by trn_agent_boot)."""
    import contextlib
    import ctypes
    import sys
    import types

    if "antenv.axon_hooks" in sys.modules:
        return
    so_path = "/opt/axon/libaxon_pjrt.so"
    mod = types.ModuleType("antenv.axon_hooks")
    state = {"hook": None}
    mod.set_axon_ntff_profile_hook = lambda h: state.__setitem__("hook", h)
    mod.get_axon_ntff_profile_hook = lambda: state["hook"]
    sys.modules["antenv.axon_hooks"] = mod
    import antenv
    antenv.axon_hooks = mod

    lib = ctypes.CDLL(so_path)
    if not hasattr(lib, "axon_start_nrt_profile"):
        return
    lib.axon_start_nrt_profile.argtypes = [ctypes.POINTER(ctypes.c_int64),
                                           ctypes.c_size_t]
    lib.axon_start_nrt_profile.restype = ctypes.c_int64
    lib.axon_stop_nrt_profile.argtypes = [ctypes.c_char_p]
    lib.axon_stop_nrt_profile.restype = ctypes.c_int64

    @contextlib.contextmanager
    def _hook_cm(output_dir, device_ids):
        import jax
        jax.devices()
        if device_ids:
            ids = (ctypes.c_int64 * len(device_ids))(*device_ids)
            rc = lib.axon_start_nrt_profile(ids, len(device_ids))
        else:
            rc = lib.axon_start_nrt_profile(None, 0)
        if rc != 0:
            raise RuntimeError(f"axon_start_nrt_profile rc={rc}")
        try:
            yield
        finally:
            lib.axon_stop_nrt_profile(str(output_dir).encode())

    mod.set_axon_ntff_profile_hook(_hook_cm)

    import concourse.bass_utils as _bu
    _bu.upload_artifacts = lambda tmpdir: f"local:{tmpdir}"


# revision 6
# speedup vs baseline: 1.0866x; 1.0866x over previous
"""Trainium2 Bass kernel for MeshNN 1-D FEM interpolation (nn_MeshNN dense_mlp).

Math: the reference builds a dense [N, 510] hat-function matrix and does a
matvec, plus Dirichlet boundary hats.  On the uniform grid every evaluation
point x lies in exactly one cell [c_k, c_{k+1}] and only the two hats centered
at c_k and c_{k+1} are nonzero there, with values (1-t) and t,
t = x/h - k, k = floor(x/h).  With w = [dd0, nodal_values..., ddL] (hat
coefficients indexed by center node) the result is exactly

    u(x) = w[k] + t*(w[k+1] - w[k])

so the per-point work is one 8-byte table lookup plus a 3-op lerp -- no dense
[N, 512] intermediate at all.

Device mapping (data-parallel across 8 NeuronCores, 16384 points each; inside
each NC the 8 GPSIMD Q7 cores each own 2048 points):
  1. x loaded twice: "wrapped" [128, 128] layout for index computation
     (point j*2048 + s*16 + q at partition 16j+q, column s -- the layout
     ap_gather wants its per-core index stream in), and "replicated" layout
     (core j's 2048 points along the free dim of partition 16j).
  2. k = floor(x*inv_h) via t = mod(y, 1); k = y - t (rounding-mode free),
     cast to int16.
  3. GPSIMD ap_gather pulls (w[k], dw[k]) pairs from a per-partition copy of
     the 512x2 table (SBUF-local gather, 4 chunks for pipelining).
  4. DVE lerp u = w0 + t*dw in the replicated layout; one DMA stores each
     Q7 core's row back to DRAM.
The (w, dw) table is host-marshalled from the small replicated input vectors
(O(n_nodes) restacking, no O(N) host work).
"""

import os

import numpy as np

import concourse.bacc as bacc
import concourse.bass as bass
import concourse.tile as tile
from concourse import mybir
from concourse.bass_utils import run_bass_kernel_spmd

N_POINTS = 131072
N_NODES = 512
N_CORES = 8
PPC = N_POINTS // N_CORES   # points per NeuronCore = 16384
P = 128                     # SBUF partitions
PPQ = PPC // 8              # points per Q7 core = 2048
N_CHUNKS = 4
IC = PPQ // N_CHUNKS        # gather idxs per chunk per Q7 core = 512


def _groups(ap):
    """[128, F] tile AP -> [8, F] view on partitions {0,16,...,112}."""
    return ap.rearrange("(j r) f -> j r f", r=16)[:, 0, :]


def _build_nc(inv_h: float):
    nc = bacc.Bacc("TRN2", target_bir_lowering=False, debug=False,
                   num_devices=N_CORES)
    x_d = nc.dram_tensor("x", [PPC], mybir.dt.float32,
                         kind="ExternalInput").ap()
    wp_d = nc.dram_tensor("wp", [N_NODES, 2], mybir.dt.float32,
                          kind="ExternalInput").ap()
    out_d = nc.dram_tensor("out", [PPC], mybir.dt.float32,
                           kind="ExternalOutput").ap()

    MAGIC = 8388608.0  # 2^23: adding+subtracting rounds f32 to int (RNE)
    with tile.TileContext(nc) as tc:
        with tc.tile_pool(name="sb", bufs=1) as pool:
            # --- wrapped-layout index chain first: the gathers depend on it ---
            xw = pool.tile([P, PPQ // 16], mybir.dt.float32, tag="xw")
            for j in range(8):
                eng = [nc.sync, nc.scalar][j % 2]
                eng.dma_start(
                    xw[16 * j:16 * (j + 1), :],
                    x_d[j * PPQ:(j + 1) * PPQ].rearrange("(s q) -> q s", q=16))
            yw = pool.tile([P, PPQ // 16], mybir.dt.float32, tag="yw")
            nc.scalar.activation(yw[:], xw[:],
                                 mybir.ActivationFunctionType.Copy,
                                 scale=inv_h)
            rw1 = pool.tile([P, PPQ // 16], mybir.dt.float32, tag="rw1")
            nc.scalar.activation(rw1[:], xw[:],
                                 mybir.ActivationFunctionType.Copy,
                                 bias=MAGIC, scale=inv_h)
            rw = pool.tile([P, PPQ // 16], mybir.dt.float32, tag="rw")
            nc.vector.tensor_scalar(rw[:], rw1[:], MAGIC, None,
                                    mybir.AluOpType.subtract)
            cw = pool.tile([P, PPQ // 16], mybir.dt.float32, tag="cw")
            nc.vector.tensor_tensor(cw[:], rw[:], yw[:],
                                    mybir.AluOpType.is_gt)
            kwf = pool.tile([P, PPQ // 16], mybir.dt.float32, tag="kwf")
            nc.vector.tensor_sub(kwf[:], rw[:], cw[:])
            kw = pool.tile([P, PPQ // 16], mybir.dt.int16, tag="kw")
            nc.vector.tensor_copy(kw[:], kwf[:])

            # (w, dw) table broadcast to every partition (PE queue: it's idle)
            tabsb = pool.tile([P, N_NODES * 2], mybir.dt.float32, tag="tab")
            nc.scalar.dma_start(
                tabsb[:],
                wp_d.rearrange("n d -> (n d)")[None, :]
                    .to_broadcast([P, N_NODES * 2]),
            )
            # replicated layout: core j's points on partition 16j; other rows
            # zero-filled (their lanes compute garbage that is never stored)
            x2 = pool.tile([P, PPQ], mybir.dt.float32, tag="x2")
            nc.scalar.memset(x2[:], 0.0)
            nc.sync.dma_start(
                x2[:].rearrange("(j r) f -> j r f", r=16)[:, 0, :],
                x_d.rearrange("(j f) -> j f", j=8))

            y2 = pool.tile([P, PPQ], mybir.dt.float32, tag="y2")
            nc.scalar.activation(y2[:], x2[:],
                                 mybir.ActivationFunctionType.Copy,
                                 scale=inv_h)
            r1 = pool.tile([P, PPQ], mybir.dt.float32, tag="r1")
            nc.scalar.activation(r1[:], x2[:],
                                 mybir.ActivationFunctionType.Copy,
                                 bias=MAGIC, scale=inv_h)
            r = pool.tile([P, PPQ], mybir.dt.float32, tag="r")
            nc.vector.tensor_scalar(r[:], r1[:], MAGIC, None,
                                    mybir.AluOpType.subtract)
            cmp = pool.tile([P, PPQ], mybir.dt.float32, tag="cmp")
            nc.vector.tensor_tensor(cmp[:], r[:], y2[:],
                                    mybir.AluOpType.is_gt)
            t0 = pool.tile([P, PPQ], mybir.dt.float32, tag="t0")
            nc.vector.tensor_sub(t0[:], y2[:], r[:])
            t = pool.tile([P, PPQ], mybir.dt.float32, tag="t")
            nc.vector.tensor_add(t[:], t0[:], cmp[:])

            u = pool.tile([P, PPQ], mybir.dt.float32, tag="u")
            for c in range(N_CHUNKS):
                isl = slice(c * IC, (c + 1) * IC)
                ssl = slice(c * (IC // 16), (c + 1) * (IC // 16))
                g = pool.tile([P, IC * 2], mybir.dt.float32, tag=f"g{c % 2}")
                nc.gpsimd.ap_gather(
                    out_ap=g[:].rearrange("p (i d) -> p i d", d=2),
                    in_ap=tabsb[:].rearrange("p (n d) -> p n d", d=2),
                    idxs_ap=kw[:, ssl],
                    channels=P,
                    num_elems=N_NODES,
                    d=2,
                    num_idxs=IC,
                )
                g3 = g[:].rearrange("p (i d) -> p i d", d=2)
                w0 = g3[:, :, 0]
                dw = g3[:, :, 1]
                m = pool.tile([P, IC], mybir.dt.float32, tag=f"m{c % 2}")
                nc.vector.tensor_mul(m[:], t[:, isl], dw)
                nc.vector.tensor_add(u[:, isl], m[:], w0)

            nc.sync.dma_start(out_d.rearrange("(j f) -> j f", j=8),
                              _groups(u[:]))

    nc.compile()
    return nc


_CACHE = {}


def kernel(x, coordinates, nodal_values, dd_weights):
    x = np.asarray(x, dtype=np.float32)
    c = np.asarray(coordinates, dtype=np.float32)
    v = np.asarray(nodal_values, dtype=np.float32)
    dd = np.asarray(dd_weights, dtype=np.float32)

    # hat coefficients by center node: [dd0, v..., ddL]; table rows (w, dw)
    w = np.empty(N_NODES, dtype=np.float32)
    w[0] = dd[0]
    w[1:N_NODES - 1] = v
    w[N_NODES - 1] = dd[1]
    wp = np.zeros((N_NODES, 2), dtype=np.float32)
    wp[:, 0] = w
    wp[:N_NODES - 1, 1] = w[1:] - w[:N_NODES - 1]

    inv_h = float(1.0 / (np.float32(c[1]) - np.float32(c[0])))

    if "nc" not in _CACHE:
        _CACHE["nc"] = _build_nc(inv_h)
    nc = _CACHE["nc"]

    chunks = x.reshape(N_CORES, PPC)
    in_maps = [{"x": np.ascontiguousarray(chunks[i]), "wp": wp}
               for i in range(N_CORES)]

    trace = bool(int(os.environ.get("BASS_KERNEL_TRACE", "0")))
    if trace:
        _install_ntff_hook()
    res = run_bass_kernel_spmd(nc, in_maps, list(range(N_CORES)), trace=trace)
    if trace:
        _CACHE["exec_time_ns"] = res.exec_time_ns
        _CACHE["profile_json"] = res.profile_json
        _CACHE["trace"] = res.instructions_and_trace

    out = np.concatenate([res.results[i]["out"] for i in range(N_CORES)])
    return out.reshape(N_POINTS, 1)


def _install_ntff_hook():
    """Shim antenv.axon_hooks + NTFF ctypes hook so trace=True works under
    axon in this container (normally installed by trn_agent_boot)."""
    import contextlib
    import ctypes
    import sys
    import types

    if "antenv.axon_hooks" in sys.modules:
        return
    so_path = "/opt/axon/libaxon_pjrt.so"
    mod = types.ModuleType("antenv.axon_hooks")
    state = {"hook": None}
    mod.set_axon_ntff_profile_hook = lambda h: state.__setitem__("hook", h)
    mod.get_axon_ntff_profile_hook = lambda: state["hook"]
    sys.modules["antenv.axon_hooks"] = mod
    import antenv
    antenv.axon_hooks = mod

    lib = ctypes.CDLL(so_path)
    if not hasattr(lib, "axon_start_nrt_profile"):
        return
    lib.axon_start_nrt_profile.argtypes = [ctypes.POINTER(ctypes.c_int64),
                                           ctypes.c_size_t]
    lib.axon_start_nrt_profile.restype = ctypes.c_int64
    lib.axon_stop_nrt_profile.argtypes = [ctypes.c_char_p]
    lib.axon_stop_nrt_profile.restype = ctypes.c_int64

    @contextlib.contextmanager
    def _hook_cm(output_dir, device_ids):
        import jax
        jax.devices()
        if device_ids:
            ids = (ctypes.c_int64 * len(device_ids))(*device_ids)
            rc = lib.axon_start_nrt_profile(ids, len(device_ids))
        else:
            rc = lib.axon_start_nrt_profile(None, 0)
        if rc != 0:
            raise RuntimeError(f"axon_start_nrt_profile rc={rc}")
        try:
            yield
        finally:
            lib.axon_stop_nrt_profile(str(output_dir).encode())

    mod.set_axon_ntff_profile_hook(_hook_cm)

    import concourse.bass_utils as _bu
    _bu.upload_artifacts = lambda tmpdir: f"local:{tmpdir}"
